# revision 1
# baseline (speedup 1.0000x reference)
"""Bass/Trainium2 kernel for a 2-layer GAT (nn_GAT_59115929862612) — v3.

Design (8 NeuronCores, SPMD single NEFF):
- Edge parallelism by src-node ownership: core c owns global nodes
  [c*6250, (c+1)*6250) and processes every edge whose src lies there.
  Segment-sums over src => each core produces complete rows for its nodes.
- GLOBAL node-order tables (no per-core rotation): dst gather indices are
  identical on every core => ONE shared edge-stream set for BOTH layers.
- bf16 feature rows: table1 row = [asrc1 4 | adst1 4 | feat 256] bf16 in a
  384-halfword (768B) row; table2/g2 row = [asrc2 2 | adst2 2 | feat 242]
  bf16 in a 256-hw (512B) row. dma_gather elem must be a 256B multiple.
- dma_gather cost on TRN2 is ~9 ns PER INDEX (Q7 descriptor emission),
  nearly independent of element size -- so the design minimizes gather
  indices: ONE gathered row per edge per layer (a_dst rides in the row),
  and a_src is delivered with NO gather at all: St = transpose(S) via
  TensorE, then aedge = St^T @ asw broadcasts the window's per-node a_src
  (SBUF-resident asw tables) to edge slots.
- Per-window exact gather counts (num_idxs_reg = max-over-cores count,
  trailing -1 indices skipped by the DMA) minimize padded gather traffic.
  Pad slots read zeros (last gather column pre-zeroed) so reused buffers
  never accumulate exp() garbage.
- One-hot S (bf16) segment-sum matmuls accumulate [p | p*feat] into PSUM
  f32; the layer-1 epilogue fuses the layer-2 feature transform (transpose
  + W2aug matmuls) and writes g2loc directly; AllGather shares table2.
- Software-pipelined window loop: gathers run LAG=2 windows ahead;
  S/St/aedge prep (stream-only deps) overlaps gather emission.

Numerics: bf16 storage everywhere was validated on host vs f64 reference:
rel err ~6e-3 (gate 2e-2). exp() without global max-shift is safe (|e|<~25).
"""

import math
import os

import numpy as np

N_NODES = 50000
N_EDGES = 800000
N_CORES = 8
NPC = N_NODES // N_CORES
HALF = N_NODES // 2
NFEAT = 128
ALPHA = 0.2
P = 128

H1, D1 = 4, 64
H2, D2 = 2, 121
F1 = H1 * D1                      # 256
F2 = H2 * D2                      # 242
R1C = 2 * H1 + F1                 # 264 used cols in table1 row
R2C = 2 * H2 + F2                 # 246 used cols in table2 row
ROW1H = 384                       # table1 row halfwords (768B)
ROW2H = 256                       # table2 row halfwords (512B)
AROWH = 128                       # asrc1loc row halfwords (256B)
NWIN = math.ceil(NPC / P)         # 49


def _bf16(x):
    import ml_dtypes
    return np.asarray(x, np.float32).astype(ml_dtypes.bfloat16)


# ---------------------------------------------------------------- host prep

def _prep_weights(W1, a1, b1, W2, a2, b2):
    W1 = np.asarray(W1, np.float32)
    a1 = np.asarray(a1, np.float32)
    W2 = np.asarray(W2, np.float32)
    a2 = np.asarray(a2, np.float32)
    w1aug = np.zeros((NFEAT, R1C), np.float32)
    for h in range(H1):
        w1aug[:, h] = W1[h] @ a1[h, :D1, 0]
        w1aug[:, H1 + h] = W1[h] @ a1[h, D1:, 0]
        w1aug[:, 2 * H1 + h * D1:2 * H1 + (h + 1) * D1] = W1[h]
    w2full = np.zeros((F1, R2C), np.float32)
    for h in range(H2):
        w2full[:, h] = W2[h] @ a2[h, :D2, 0]
        w2full[:, H2 + h] = W2[h] @ a2[h, D2:, 0]
        w2full[:, 2 * H2 + h * D2:2 * H2 + (h + 1) * D2] = W2[h]
    w2aug = np.stack([w2full[:NFEAT], w2full[NFEAT:]])      # [2, 128, 246]
    b1b = np.tile(np.asarray(b1, np.float32).reshape(1, F1), (P, 1)).copy()
    b2 = np.asarray(b2, np.float32)
    b2m = np.tile((0.5 * (b2[0] + b2[1])).reshape(1, D2), (P, 1)).copy()
    return _bf16(w1aug), _bf16(w2aug), b1b, b2m


def _pack_idx(flat):
    """[num] int (num % 16 == 0) -> [128, num//16] int16 wrap layout."""
    num = flat.shape[0]
    a = flat.reshape(num // 16, 16).T.astype(np.int16)
    return np.tile(a, (8, 1))


def _prep_edges(edge_list):
    """Shared (both layers) per-core edge streams, global dst order.

    Returns (streams[c] dict, meta) where meta has per-window static info:
    tlo[w], thi[w] (slot columns), clo[w], chi[w] (exact gather counts,
    maxed over cores), offlo[w], offhi[w], offsr[w] (halfword offsets into
    the resident stream tiles), TMAX.
    """
    src = np.asarray(edge_list[0], np.int64)
    dst = np.asarray(edge_list[1], np.int64)

    percore = []
    nlo = np.zeros((N_CORES, NWIN), int)
    nhi = np.zeros((N_CORES, NWIN), int)
    for c in range(N_CORES):
        base = c * NPC
        sel = (src >= base) & (src < base + NPC)
        sl = (src[sel] - base).astype(np.int32)
        dg = dst[sel].astype(np.int32)
        order = np.argsort(sl, kind="stable")
        sl, dg = sl[order], dg[order]
        win = sl >> 7
        bounds = np.searchsorted(win, np.arange(NWIN + 1))
        wins = []
        for w in range(NWIN):
            s, e = bounds[w], bounds[w + 1]
            srel_w = (sl[s:e] - w * P).astype(np.float32)
            slo_w = sl[s:e]
            d_w = dg[s:e]
            lo = d_w < HALF
            nlo[c, w] = int(lo.sum())
            nhi[c, w] = int((~lo).sum())
            wins.append((srel_w[lo], d_w[lo], slo_w[lo],
                         srel_w[~lo], d_w[~lo] - HALF, slo_w[~lo]))
        percore.append(wins)

    clo = nlo.max(axis=0)
    chi = nhi.max(axis=0)
    tlo = np.maximum(1, np.ceil(clo / P).astype(int))
    thi = np.maximum(1, np.ceil(chi / P).astype(int))
    TMAX = int((tlo + thi).max())
    offlo = np.concatenate([[0], np.cumsum(tlo * 8)])
    offhi = np.concatenate([[0], np.cumsum(thi * 8)])
    offsr = np.concatenate([[0], np.cumsum(tlo + thi)])

    streams = []
    for c in range(N_CORES):
        ilo = np.zeros((P, offlo[-1]), np.int16)
        ihi = np.zeros((P, offhi[-1]), np.int16)
        srl = np.zeros((P, offsr[-1]), np.float32)
        for w in range(NWIN):
            sr_lo, d_lo, s_lo, sr_hi, d_hi, s_hi = percore[c][w]

            def mk(nreal, cnt, tcols, dvals):
                nslots = tcols * P
                di = np.full(nslots, -1, np.int32)
                di[:nreal] = dvals
                if cnt > nreal:                    # 0-pads to match count
                    di[nreal:cnt] = 0
                return _pack_idx(di)

            ilo[:, offlo[w]:offlo[w + 1]] = mk(len(d_lo), clo[w], tlo[w], d_lo)
            ihi[:, offhi[w]:offhi[w + 1]] = mk(len(d_hi), chi[w], thi[w], d_hi)

            # srel grid [P, tlo+thi]: col-major slot fill, -1 elsewhere
            g = np.full((tlo[w] + thi[w], P), -1.0, np.float32)
            gl = g[:tlo[w]].reshape(-1)
            gl[:len(sr_lo)] = sr_lo
            gh = g[tlo[w]:].reshape(-1)
            gh[:len(sr_hi)] = sr_hi
            srl[:, offsr[w]:offsr[w + 1]] = g.T
        streams.append(dict(ilo=ilo, ihi=ihi, srl=srl))

    meta = dict(tlo=tlo.tolist(), thi=thi.tolist(),
                clo=clo.tolist(), chi=chi.tolist(),
                offlo=offlo.tolist(), offhi=offhi.tolist(),
                offsr=offsr.tolist(), TMAX=TMAX)
    return streams, meta


# ---------------------------------------------------------------- program

def _build_program(meta, phases="ABDE", repeats=1, debug_g2=False):
    import concourse.bacc as bacc
    import concourse.mybir as mybir
    import concourse.tile as tile
    from concourse.masks import make_identity

    f32 = mybir.dt.float32
    f32r = mybir.dt.float32r
    bf16 = mybir.dt.bfloat16
    i16 = mybir.dt.int16
    AF = mybir.ActivationFunctionType
    OP = mybir.AluOpType

    tlo, thi = meta["tlo"], meta["thi"]
    clo, chi = meta["clo"], meta["chi"]
    offlo, offhi, offsr = meta["offlo"], meta["offhi"], meta["offsr"]
    TMAX = meta["TMAX"]
    n = N_NODES

    nc = bacc.Bacc("TRN2", target_bir_lowering=False, debug=False,
                   num_devices=N_CORES)

    xT = nc.dram_tensor("xT", [NFEAT, n], bf16, kind="ExternalInput")
    xTo = nc.dram_tensor("xTo", [NFEAT, NPC], bf16, kind="ExternalInput")
    w1aug = nc.dram_tensor("w1aug", [NFEAT, R1C], bf16, kind="ExternalInput")
    w2aug = nc.dram_tensor("w2aug", [2, NFEAT, R2C], bf16, kind="ExternalInput")
    b1b = nc.dram_tensor("b1b", [P, F1], f32, kind="ExternalInput")
    b2m = nc.dram_tensor("b2m", [P, D2], f32, kind="ExternalInput")
    iota_in = nc.dram_tensor("iota", [P, P], f32, kind="ExternalInput")
    ilo_t = nc.dram_tensor("ilo", [P, offlo[-1]], i16, kind="ExternalInput")
    ihi_t = nc.dram_tensor("ihi", [P, offhi[-1]], i16, kind="ExternalInput")
    srl_t = nc.dram_tensor("srl", [P, offsr[-1]], f32, kind="ExternalInput")
    out = nc.dram_tensor("out", [NPC, D2], f32, kind="ExternalOutput")

    table1 = nc.dram_tensor("table1", [n, ROW1H], bf16, kind="Internal")
    g2loc = nc.dram_tensor("g2loc", [NPC, ROW2H], bf16,
                           kind="ExternalOutput" if debug_g2 else "Internal")
    table2 = nc.dram_tensor("table2", [n, ROW2H], bf16, kind="Internal",
                            addr_space="Shared")

    with tile.TileContext(nc) as tc:
      with (
            tc.tile_pool(name="const", bufs=1) as cpool,
            tc.tile_pool(name="hphase", bufs=2) as hpool,
            tc.tile_pool(name="hpsum", bufs=2, space="PSUM") as hpsum,
            tc.tile_pool(name="edge", bufs=3) as epool,
            tc.tile_pool(name="epsum", bufs=3, space="PSUM") as epsum,
            tc.tile_pool(name="cpsum", bufs=1, space="PSUM") as cpsum,
            tc.tile_pool(name="epi", bufs=2) as ipool,
      ):
        for _rep in range(repeats):
            # ---- constants + resident streams
            w1sb = cpool.tile([NFEAT, R1C], bf16)
            nc.sync.dma_start(w1sb[:], w1aug[:])
            w2sb = cpool.tile([NFEAT, 2 * R2C], bf16)
            nc.sync.dma_start(w2sb[:, 0:R2C], w2aug[0])
            nc.sync.dma_start(w2sb[:, R2C:2 * R2C], w2aug[1])
            b1sb = cpool.tile([P, F1], f32)
            nc.sync.dma_start(b1sb[:], b1b[:])
            b2sb = cpool.tile([P, D2], f32)
            nc.sync.dma_start(b2sb[:], b2m[:])
            iotasb = cpool.tile([P, P], f32)
            nc.sync.dma_start(iotasb[:], iota_in[:])
            idsb = cpool.tile([P, P], bf16)
            make_identity(nc, idsb[:])
            silo = cpool.tile([P, offlo[-1]], i16)
            nc.sync.dma_start(silo[:], ilo_t[:])
            sihi = cpool.tile([P, offhi[-1]], i16)
            nc.sync.dma_start(sihi[:], ihi_t[:])
            asw1all = cpool.tile([P, NWIN * H1], bf16)
            nc.vector.memset(asw1all[:], 0.0)
            asw2all = cpool.tile([P, NWIN * H2], bf16)
            nc.vector.memset(asw2all[:], 0.0)
            ssrl = cpool.tile([P, offsr[-1]], f32)
            nc.sync.dma_start(ssrl[:], srl_t[:])

            # ---- phase A: table1 rows for all n nodes. Full-ROW1H rows are
            # written (junk pad cols) so the store is one contiguous run.
            CH = 4096
            for i in range(math.ceil(n / CH) if "A" in phases else 0):
                n0 = i * CH
                m = min(CH, n - n0)
                nsub = math.ceil(m / P)
                nfull = m // P
                xt = hpool.tile([NFEAT, CH], bf16, tag="xt")
                nc.sync.dma_start(xt[:, :m], xT[:, n0:n0 + m])
                sbA = hpool.tile([P, CH // P, ROW1H], bf16, tag="sbA")
                for j in range(nsub):
                    nn = min(P, m - j * P)
                    psA = hpsum.tile([P, R1C], f32, tag="psA", bufs=2)
                    nc.tensor.matmul(
                        psA[:nn, :], lhsT=xt[:, j * P:j * P + nn],
                        rhs=w1sb[:], start=True, stop=True)
                    if j % 2 == 0:
                        nc.scalar.copy(sbA[:nn, j, 0:R1C], psA[:nn, :])
                    else:
                        nc.vector.tensor_copy(sbA[:nn, j, 0:R1C], psA[:nn, :])
                if nfull:
                    dstp = table1[n0:n0 + nfull * P, :].rearrange(
                        "(j p) c -> p j c", p=P)
                    nc.sync.dma_start(dstp, sbA[:, 0:nfull, :])
                if nfull < nsub:
                    nn = m - nfull * P
                    nc.sync.dma_start(
                        table1[n0 + nfull * P:n0 + m, :],
                        sbA[:nn, nfull, :])

            # ---- phase A2: asrc1 for owned nodes -> resident asw1all
            if "A" in phases:
                xto = cpool.tile([NFEAT, NPC], bf16)
                nc.sync.dma_start(xto[:], xTo[:])
            for k in range(NWIN if "A" in phases else 0):
                n0 = k * P
                nn = min(P, NPC - n0)
                psB = cpsum.tile([P, TMAX * H1], f32, tag="psE", bufs=2)
                nc.tensor.matmul(
                    psB[:nn, 0:H1], lhsT=xto[:, n0:n0 + nn],
                    rhs=w1sb[:, 0:H1], start=True, stop=True)
                nc.scalar.copy(asw1all[:nn, k * H1:(k + 1) * H1],
                               psB[:nn, 0:H1])

            # ---- edge phase (both layers)
            def edge_layer(layer, table, aswall, ROWH, H, F, RC, epilogue,
                           whook=None):
                LAG = 2
                dtiles, stiles, petiles = {}, {}, {}

                def stage_gather(w):
                    tl, th = tlo[w], thi[w]
                    T = tl + th
                    D = epool.tile([P, TMAX, ROWH], bf16, tag="D", bufs=4)
                    # zero the final column of each gather region: trailing
                    # skipped pad slots live there and must read as 0 (adst=0,
                    # feat=0) so they contribute nothing and never overflow
                    if clo[w] < tl * P:
                        nc.vector.memset(D[:, tl - 1, :], 0.0)
                    if chi[w] < th * P:
                        nc.vector.memset(D[:, T - 1, :], 0.0)
                    nc.gpsimd.dma_gather(
                        out_ap=D[:, 0:tl, :], in_ap=table[0:HALF, :],
                        idxs_ap=silo[:, offlo[w]:offlo[w + 1]],
                        num_idxs=tl * P, num_idxs_reg=clo[w],
                        elem_size=ROWH, single_packet=False)
                    nc.gpsimd.dma_gather(
                        out_ap=D[:, tl:T, :], in_ap=table[HALF:, :],
                        idxs_ap=sihi[:, offhi[w]:offhi[w + 1]],
                        num_idxs=th * P, num_idxs_reg=chi[w],
                        elem_size=ROWH, single_packet=False)
                    dtiles[w] = D

                def stage_prep(w):
                    # S / St / aedge depend only on resident streams + asw
                    T = tlo[w] + thi[w]
                    S = epool.tile([P, TMAX, P], bf16, tag="S", bufs=4)
                    nc.vector.tensor_tensor(
                        out=S[:, 0:T, :],
                        in0=iotasb[:].unsqueeze(1).broadcast_to([P, T, P]),
                        in1=ssrl[:, offsr[w]:offsr[w + 1]].unsqueeze(2)
                            .broadcast_to([P, T, P]),
                        op=OP.is_equal)
                    St = epool.tile([P, TMAX, P], bf16, tag="St", bufs=4)
                    for t in range(T):
                        psT = cpsum.tile([P, P], bf16, tag="psT", bufs=2)
                        nc.tensor.transpose(psT[:], S[:, t, :], idsb[:])
                        if t % 2 == 0:
                            nc.scalar.copy(St[:, t, :], psT[:])
                        else:
                            nc.vector.tensor_copy(St[:, t, :], psT[:])
                    psE = cpsum.tile([P, TMAX * H1], f32, tag="psE", bufs=2)
                    asw = aswall[:, w * H:(w + 1) * H]
                    for t in range(T):
                        nc.tensor.matmul(psE[:, t * H:(t + 1) * H],
                                         lhsT=St[:, t, :], rhs=asw,
                                         start=True, stop=True)
                    stiles[w] = S
                    petiles[w] = psE

                def stage_consume(w):
                    NW = min(P, NPC - w * P)
                    T = tlo[w] + thi[w]
                    D, S, psE = dtiles.pop(w), stiles.pop(w), petiles.pop(w)
                    # e = lrelu(asrc[src] + adst[dst]); p = exp(e)
                    et = epool.tile([P, TMAX, H1], f32, tag="et", bufs=2)
                    nc.vector.tensor_tensor(
                        out=et[:, 0:T, 0:H],
                        in0=psE[:, 0:T * H].rearrange("p (t h) -> p t h", h=H),
                        in1=D[:, 0:T, H:2 * H], op=OP.add)
                    et2 = epool.tile([P, TMAX, H1], f32, tag="et2", bufs=2)
                    nc.vector.tensor_scalar_mul(et2[:, 0:T, 0:H],
                                                et[:, 0:T, 0:H], ALPHA)
                    nc.vector.tensor_tensor(
                        out=et[:, 0:T, 0:H], in0=et[:, 0:T, 0:H],
                        in1=et2[:, 0:T, 0:H], op=OP.max)
                    nc.scalar.activation(D[:, 0:T, H:2 * H], et[:, 0:T, 0:H],
                                         AF.Exp)
                    # feat *= p (per-head broadcast)
                    feat = D[:, 0:T, 2 * H:2 * H + F].rearrange(
                        "p t (h d) -> p t h d", h=H)
                    pb = D[:, 0:T, H:2 * H].unsqueeze(3).broadcast_to(
                        [P, T, H, F // H])
                    nc.vector.tensor_tensor(out=feat, in0=feat, in1=pb,
                                            op=OP.mult)
                    ps = epsum.tile([P, H1 + F1], f32, tag="ps", bufs=2)
                    for t in range(T):
                        nc.tensor.matmul(ps[:, 0:H + F], lhsT=S[:, t, :],
                                         rhs=D[:, t, H:2 * H + F],
                                         start=(t == 0), stop=(t == T - 1))
                    epilogue(ps, w, NW)

                for w in range(NWIN + LAG):
                    if w < NWIN:
                        stage_gather(w)
                        stage_prep(w)
                    if w >= LAG:
                        stage_consume(w - LAG)
                    if whook is not None:
                        whook(w)

            # ---- layer-1 epilogue: h2 = elu(hp/denom + b1); fused layer-2
            # feature transform; writes g2loc rows [asrc2|adst2|feat2]
            def epi1(ps, w, NW):
                dn = ipool.tile([P, H1], f32, tag="dn")
                nc.vector.tensor_scalar_add(dn[:, :], ps[:, 0:H1], 1e-30)
                rr = ipool.tile([P, H1], f32, tag="rr")
                nc.vector.reciprocal(rr[:, :], dn[:, :])
                hb = ipool.tile([P, F1], f32, tag="hb")
                for h in range(H1):
                    nc.vector.tensor_scalar(
                        out=hb[:, h * D1:(h + 1) * D1],
                        in0=ps[:, H1 + h * D1:H1 + (h + 1) * D1],
                        scalar1=rr[:, h:h + 1], scalar2=None, op0=OP.mult)
                nc.vector.tensor_tensor(out=hb[:], in0=hb[:], in1=b1sb[:],
                                        op=OP.add)
                mn = ipool.tile([P, F1], f32, tag="mn")
                nc.vector.tensor_scalar_min(mn[:], hb[:], 0.0)
                ex = ipool.tile([P, F1], f32, tag="ex")
                nc.scalar.activation(ex[:], mn[:], AF.Exp)
                nc.vector.tensor_scalar_add(ex[:], ex[:], -1.0)
                nc.vector.tensor_scalar_max(hb[:], hb[:], 0.0)
                h2 = ipool.tile([P, F1], bf16, tag="h2")
                nc.vector.tensor_tensor(out=h2[:], in0=hb[:], in1=ex[:],
                                        op=OP.add)
                hT = ipool.tile([P, 2, P], bf16, tag="hT")
                for k in range(2):
                    psT = cpsum.tile([P, P], bf16, tag="psT", bufs=2)
                    nc.tensor.transpose(psT[:], h2[:, k * P:(k + 1) * P],
                                        idsb[:])
                    nc.scalar.copy(hT[:, k, :], psT[:])
                ps2 = epsum.tile([P, H1 + F1], f32, tag="ps", bufs=2)
                nc.tensor.matmul(ps2[:, 0:R2C], lhsT=hT[:, 0, :],
                                 rhs=w2sb[:, 0:R2C], start=True, stop=False)
                nc.tensor.matmul(ps2[:, 0:R2C], lhsT=hT[:, 1, :],
                                 rhs=w2sb[:, R2C:2 * R2C],
                                 start=False, stop=True)
                nc.scalar.copy(asw2all[:, w * H2:(w + 1) * H2], ps2[:, 0:H2])
                g2sb = ipool.tile([P, ROW2H], bf16, tag="g2sb")
                nc.scalar.copy(g2sb[:, 0:R2C], ps2[:, 0:R2C])
                nc.sync.dma_start(g2loc[w * P:w * P + NW, :], g2sb[:NW, :])

            # chunked AllGather: share g2 rows while layer 1 still runs.
            # chunk k covers windows [7k, 7k+7); issued once those windows'
            # epilogues are safely behind the pipeline (at w = 7k+7+LAG+1).
            CW = 7
            t2view = table2[:].rearrange("(r x) c -> r x c", r=N_CORES)

            def issue_chunk(k):
                s0 = k * CW * P
                e0 = min(NPC, (k + 1) * CW * P)
                nc.gpsimd.collective_compute(
                    "AllGather", mybir.AluOpType.bypass,
                    replica_groups=[list(range(N_CORES))],
                    ins=[g2loc[s0:e0, :].opt()],
                    outs=[t2view[:, s0:e0, :].opt()])

            issued = set()

            def b_hook(w):
                if True:   # chunked AllGather disabled: strided collective
                    return  # output APs fail BIR verification on this stack
                k = (w - 3) // CW - 1
                if 0 <= k < math.ceil(NWIN / CW) and k not in issued:
                    issued.add(k)
                    issue_chunk(k)

            if "B" in phases:
                edge_layer(1, table1, asw1all, ROW1H, H1, F1, R1C, epi1,
                           whook=b_hook)

            # ---- phase D: share g2 across cores
            if "D" in phases:
                nc.gpsimd.collective_compute(
                    "AllGather", mybir.AluOpType.bypass,
                    replica_groups=[list(range(N_CORES))],
                    ins=[g2loc[:].opt()], outs=[table2[:].opt()])

            # ---- layer-2 epilogue: log_softmax(mean heads + b2) -> out
            def epi2(ps, w, NW):
                dn = ipool.tile([P, H2], f32, tag="dn2")
                nc.vector.tensor_scalar_add(dn[:, 0:H2], ps[:, 0:H2], 1e-30)
                rr = ipool.tile([P, H2], f32, tag="rr2")
                nc.vector.reciprocal(rr[:, 0:H2], dn[:, 0:H2])
                nc.vector.tensor_scalar_mul(rr[:, 0:H2], rr[:, 0:H2], 0.5)
                o = ipool.tile([P, D2], f32, tag="o")
                t1 = ipool.tile([P, D2], f32, tag="t1")
                nc.vector.tensor_scalar(
                    out=o[:], in0=ps[:, H2:H2 + D2], scalar1=rr[:, 0:1],
                    scalar2=None, op0=OP.mult)
                nc.vector.tensor_scalar(
                    out=t1[:], in0=ps[:, H2 + D2:H2 + 2 * D2],
                    scalar1=rr[:, 1:2], scalar2=None, op0=OP.mult)
                nc.vector.tensor_tensor(out=o[:], in0=o[:], in1=t1[:],
                                        op=OP.add)
                nc.vector.tensor_tensor(out=o[:], in0=o[:], in1=b2sb[:],
                                        op=OP.add)
                nmx = ipool.tile([P, 1], f32, tag="nmx")
                nc.vector.tensor_reduce(out=nmx[:], in_=o[:],
                                        axis=mybir.AxisListType.X,
                                        op=OP.max, negate=True)
                exs = ipool.tile([P, D2], f32, tag="exs")
                sm = ipool.tile([P, 1], f32, tag="sm")
                nc.scalar.activation(exs[:], o[:], AF.Exp,
                                     bias=nmx[:, 0:1], accum_out=sm[:, 0:1])
                lg = ipool.tile([P, 1], f32, tag="lg")
                nc.scalar.activation(lg[:, 0:1], sm[:, 0:1], AF.Ln)
                res = ipool.tile([P, D2], f32, tag="res")
                nc.vector.tensor_scalar(
                    out=res[:], in0=o[:], scalar1=nmx[:, 0:1],
                    scalar2=lg[:, 0:1], op0=OP.add, op1=OP.subtract)
                nc.sync.dma_start(out[w * P:w * P + NW, :], res[:NW, :])

            if "E" in phases:
                edge_layer(2, table2, asw2all, ROW2H, H2, F2, R2C, epi2)

    nc.compile()
    return nc


def _host_inputs(x, edge_list, W1, a1, b1, W2, a2, b2):
    w1aug, w2aug, b1b, b2m = _prep_weights(W1, a1, b1, W2, a2, b2)
    streams, meta = _prep_edges(edge_list)
    iota = np.tile(np.arange(P, dtype=np.float32).reshape(1, P), (P, 1)).copy()
    x = np.asarray(x, np.float32)
    xTb = _bf16(np.ascontiguousarray(x.T))
    in_maps = []
    for c in range(N_CORES):
        base = c * NPC
        m = dict(
            xT=xTb, xTo=_bf16(np.ascontiguousarray(x[base:base + NPC].T)),
            w1aug=w1aug, w2aug=w2aug, b1b=b1b, b2m=b2m, iota=iota,
            ilo=streams[c]["ilo"], ihi=streams[c]["ihi"],
            srl=streams[c]["srl"],
        )
        in_maps.append(m)
    return in_maps, meta


def kernel(x, edge_list, W1, a1, b1, W2, a2, b2):
    from concourse.bass_utils import run_bass_kernel_spmd

    in_maps, meta = _host_inputs(x, edge_list, W1, a1, b1, W2, a2, b2)
    nc = _build_program(meta)
    res = run_bass_kernel_spmd(nc, in_maps, core_ids=list(range(N_CORES)))
    return np.concatenate([res.results[c]["out"] for c in range(N_CORES)],
                          axis=0)

